# revision 9
# baseline (speedup 1.0000x reference)
"""RBM local-operator kernel for Trainium2 (8 NeuronCores, SPMD).

Math: for y_k = x with spin k flipped (x in {-1,+1}^N),
  logpsi(y_k) - logpsi(x)
    = -2 x_k a_k + sum_h [ logcosh(th_h - 2 x_k W_kh) - logcosh(th_h) ]
    = -2 x_k a_k + S1_k + sum_h log(1 - x_k t_h tau_kh)
with th = xW + b, t = tanh(th), tau = tanh(2W), S1_k = sum_h logcosh(2W_kh).
Since |t*tau| <~ 0.08, log(1-u) = -sum_n u^n / n converges in a few terms and
each term is a matmul over h:  sum_h t^n tau^n = (t^n) @ (tau^n)^T.
Also logcosh(2W) = -0.5*log(1 - tau^2) = 0.5*(v + v^2/2 + v^3/3 + ...), v=tau^2,
so S1 folds into the same PSUM accumulation via constant-valued lhsT tiles.

Sharding: hidden dim H=2048 split over 8 cores (256 each). Each core emits
  q_even = -S1 + sum_{n even} M_n / n      (plus nothing global)
  q_odd  = 2a/8 + sum_{n odd} M_n / n     (2a scaled so the 8-core sum = 2a)
Host combines: out = exp(-(sum_c q_even + x * sum_c q_odd)) @ Oxy.
"""

import sys

import numpy as np

_BASS_REPO = "/opt/trn_rl_repo"
if _BASS_REPO not in sys.path:
    sys.path.insert(0, _BASS_REPO)

from contextlib import ExitStack

import concourse.bass as bass
import concourse.tile as tile
from concourse import bacc, mybir
from concourse.bass_utils import run_bass_kernel_spmd

B, N, H, NCORES = 64, 512, 2048, 8
HL = H // NCORES          # hidden slice per core: 256
HT = HL // 128            # SBUF partition tiles per slice: 2
CCH = 5                   # theta contraction chunks: 640 = 5*128 padded rows
F32 = mybir.dt.float32
AF = mybir.ActivationFunctionType
ALU = mybir.AluOpType

_CACHE = {}


def _build_bass():
    nc = bacc.Bacc(
        "TRN2", target_bir_lowering=False, debug=False, num_devices=NCORES
    )
    wct_d = nc.declare_dram_parameter("wct", [128, HT, N], F32, isOutput=False)
    wc_d = nc.declare_dram_parameter("wc", [128, CCH, HL], F32, isOutput=False)
    xt_d = nc.declare_dram_parameter("xt", [128, CCH, B], F32, isOutput=False)
    qe_d = nc.declare_dram_parameter("q_even", [B, N], F32, isOutput=True)
    qo_d = nc.declare_dram_parameter("q_odd", [B, N], F32, isOutput=True)

    with tile.TileContext(nc) as tc, ExitStack() as ctx:
        pool = ctx.enter_context(tc.tile_pool(name="sbuf", bufs=1))
        psum = ctx.enter_context(
            tc.tile_pool(name="psum", bufs=1, space=bass.MemorySpace.PSUM)
        )

        wct = pool.tile([128, HT, N], F32, tag="wct")
        nc.sync.dma_start(wct[:], wct_d[:])
        wc = pool.tile([128, CCH, HL], F32, tag="wc")
        nc.sync.dma_start(wc[:], wc_d[:])
        xt = pool.tile([128, CCH, B], F32, tag="xt")
        nc.sync.dma_start(xt[:], xt_d[:])
        # S1 series coefficients: -(1/2, 1/4, 1/6) on tau^2, tau^4, tau^6
        cs1 = []
        for i, v in enumerate((-0.5, -0.25, -1.0 / 6.0)):
            ct = pool.tile([128, B], F32, tag=f"cs1_{i}")
            nc.gpsimd.memset(ct[:], v)
            cs1.append(ct)

        # thetaT[h, b] = sum_n W_aug[n, h] * X_aug[n, b]  (bias via augmented row)
        th = psum.tile([128, HT, B], F32, tag="th")
        for t in range(HT):
            for c in range(CCH):
                nc.tensor.matmul(
                    th[:, t, :],
                    wc[:, c, bass.ts(t, 128)],
                    xt[:, c, :],
                    start=(c == 0),
                    stop=(c == CCH - 1),
                )

        # T'_n = t^n / n  (lhsT operands, scaled so PSUM accumulates sum M_n/n)
        T1 = pool.tile([128, HT, B], F32, tag="T1")
        nc.scalar.activation(T1[:], th[:], AF.Tanh)
        T2 = pool.tile([128, HT, B], F32, tag="T2")
        nc.vector.scalar_tensor_tensor(T2[:], T1[:], 0.5, T1[:], ALU.mult, ALU.mult)
        T3 = pool.tile([128, HT, B], F32, tag="T3")
        nc.vector.scalar_tensor_tensor(
            T3[:], T2[:], 2.0 / 3.0, T1[:], ALU.mult, ALU.mult
        )
        T4 = pool.tile([128, HT, B], F32, tag="T4")
        nc.vector.tensor_mul(T4[:], T2[:], T2[:])
        T5 = pool.tile([128, HT, B], F32, tag="T5")
        nc.vector.scalar_tensor_tensor(
            T5[:], T4[:], 4.0 / 5.0, T1[:], ALU.mult, ALU.mult
        )

        # Gamma_n = tanh(2W)^n, layout [h_p, k_f]
        G1 = pool.tile([128, HT, N], F32, tag="G1")
        nc.scalar.activation(G1[:], wct[:], AF.Tanh, scale=2.0)
        G2 = pool.tile([128, HT, N], F32, tag="G2")
        nc.scalar.activation(G2[:], G1[:], AF.Square)
        G4 = pool.tile([128, HT, N], F32, tag="G4")
        nc.scalar.activation(G4[:], G2[:], AF.Square)
        G3 = pool.tile([128, HT, N], F32, tag="G3")
        nc.vector.tensor_mul(G3[:], G2[:], G1[:])
        G5 = pool.tile([128, HT, N], F32, tag="G5")
        nc.vector.tensor_mul(G5[:], G4[:], G1[:])
        G6 = pool.tile([128, HT, N], F32, tag="G6")
        nc.vector.tensor_mul(G6[:], G2[:], G4[:])

        # Odd-parity accumulation: sum_{n odd} M_n/n  (2a added host-side)
        qo = psum.tile([B, N], F32, tag="qo")
        odd_pairs = [(T1, G1), (T3, G3), (T5, G5)]
        n_odd_mm = len(odd_pairs) * HT
        k = 0
        for Tn, Gn in odd_pairs:
            for t in range(HT):
                nc.tensor.matmul(
                    qo[:],
                    Tn[:, t, :],
                    Gn[:, t, :],
                    start=(k == 0),
                    stop=(k == n_odd_mm - 1),
                )
                k += 1

        # Even-parity accumulation: sum_{n even} M_n/n - S1 (via constant lhsT)
        qe = psum.tile([B, N], F32, tag="qe")
        even_pairs = [(T2, G2), (T4, G4)]
        s1_pairs = [(cs1[0], G2), (cs1[1], G4), (cs1[2], G6)]
        k = 0
        n_even_mm = (len(even_pairs) + len(s1_pairs)) * HT
        for Tn, Gn in even_pairs:
            for t in range(HT):
                nc.tensor.matmul(
                    qe[:], Tn[:, t, :], Gn[:, t, :], start=(k == 0), stop=False
                )
                k += 1
        for Cn, Gn in s1_pairs:
            for t in range(HT):
                nc.tensor.matmul(
                    qe[:], Cn[:], Gn[:, t, :], start=False, stop=(k == n_even_mm - 1)
                )
                k += 1

        qe_sb = pool.tile([B, N], F32, tag="qe_sb")
        nc.vector.tensor_copy(qe_sb[:], qe[:])
        qo_sb = pool.tile([B, N], F32, tag="qo_sb")
        nc.scalar.copy(qo_sb[:], qo[:])
        nc.sync.dma_start(qe_d[:], qe_sb[:])
        nc.sync.dma_start(qo_d[:], qo_sb[:])

    nc.compile()
    return nc


def _get_bass():
    if "nc" not in _CACHE:
        _CACHE["nc"] = _build_bass()
    return _CACHE["nc"]


def _prep_inputs(x, W, b, a):
    """Per-core input maps. All host-side layout prep, float32."""
    x = np.asarray(x, dtype=np.float32)
    W = np.asarray(W, dtype=np.float32)
    b = np.asarray(b, dtype=np.float32)
    a = np.asarray(a, dtype=np.float32)

    xt_aug = np.zeros((CCH * 128, B), dtype=np.float32)
    xt_aug[:N] = x.T
    xt_aug[N] = 1.0
    xt = np.ascontiguousarray(
        xt_aug.reshape(CCH, 128, B).transpose(1, 0, 2)
    )  # [128, CCH, B]

    in_maps = []
    for c in range(NCORES):
        sl = slice(c * HL, (c + 1) * HL)
        Wc = W[:, sl]  # [N, HL]
        wct = np.ascontiguousarray(
            Wc.T.reshape(HT, 128, N).transpose(1, 0, 2)
        )  # [128, HT, N]; wct[p, t, k] = W[k, c*HL + t*128 + p]
        wc_aug = np.zeros((CCH * 128, HL), dtype=np.float32)
        wc_aug[:N] = Wc
        wc_aug[N] = b[sl]
        wc = np.ascontiguousarray(wc_aug.reshape(CCH, 128, HL).transpose(1, 0, 2))
        in_maps.append({"wct": wct, "wc": wc, "xt": xt})
    return in_maps


def _combine(x, a, Oxy, results):
    qe = np.zeros((B, N), dtype=np.float32)
    qo = np.zeros((B, N), dtype=np.float32)
    for r in results:
        qe += r["q_even"]
        qo += r["q_odd"]
    x = np.asarray(x, dtype=np.float32)
    a = np.asarray(a, dtype=np.float32)
    Oxy = np.asarray(Oxy, dtype=np.float32)
    E = np.exp(-(qe + x * (qo + 2.0 * a)))
    return (E @ Oxy).astype(np.float32)


def kernel(x, W, b, a, Oxy):
    nc = _get_bass()
    in_maps = _prep_inputs(x, W, b, a)
    res = run_bass_kernel_spmd(nc, in_maps, list(range(NCORES))).results
    return _combine(x, a, Oxy, res)


# revision 15
# speedup vs baseline: 1.2256x; 1.2256x over previous
"""RBM local-operator kernel for Trainium2 (8 NeuronCores, SPMD).

Math: for y_k = x with spin k flipped (x in {-1,+1}^N),
  logpsi(y_k) - logpsi(x)
    = -2 x_k a_k + sum_h [ logcosh(th_h - 2 x_k W_kh) - logcosh(th_h) ]
    = -2 x_k a_k + S1_k + sum_h log(1 - x_k t_h tau_kh)
with th = xW + b, t = tanh(th), tau = tanh(2W), S1_k = sum_h logcosh(2W_kh).
Since |t*tau| <~ 0.08, log(1-u) = -sum_n u^n / n converges in 4 terms and each
term is a matmul over h:  sum_h t^n tau^n = (t^n) @ (tau^n)^T.  Also
logcosh(2W) = -0.5*log(1 - tau^2) ~= 0.5*(v + v^2/2), v = tau^2, which folds
into the same PSUM accumulation via a constant (-0.5) lhsT tile.

Sharding: hidden dim H=2048 split over 8 cores (256 each). Each core emits
  q_even = -S1 + sum_{n in 2,4} M_n / n
  q_odd  =       sum_{n in 1,3} M_n / n
Host combines: out = exp(-(sum_c q_even + x * (sum_c q_odd + 2a))) @ Oxy.

Matmuls run as float32r (single-pass fp32, 4x faster than fp32 for moving
dim >= 256) via bitcast views of the fp32 tiles.
"""

import sys

import numpy as np

_BASS_REPO = "/opt/trn_rl_repo"
if _BASS_REPO not in sys.path:
    sys.path.insert(0, _BASS_REPO)

from contextlib import ExitStack

import concourse.bass as bass
import concourse.tile as tile
from concourse import bacc, mybir
from concourse.bass_utils import run_bass_kernel_spmd

B, N, H, NCORES = 64, 512, 2048, 8
HL = H // NCORES          # hidden slice per core: 256
HT = HL // 128            # SBUF partition tiles per slice: 2
CCH = N // 128            # theta contraction chunks: 4
F32 = mybir.dt.float32
F32R = mybir.dt.float32r
AF = mybir.ActivationFunctionType
ALU = mybir.AluOpType

_CACHE = {}


def _r(ap):
    """View an fp32 AP as float32r for single-pass PE matmul."""
    return ap.bitcast(F32R)


def _build_bass():
    nc = bacc.Bacc(
        "TRN2", target_bir_lowering=False, debug=False, num_devices=NCORES
    )
    wct_d = nc.declare_dram_parameter("wct", [128, HT, N], F32, isOutput=False)
    wc_d = nc.declare_dram_parameter("wc", [128, CCH, HL], F32, isOutput=False)
    xt_d = nc.declare_dram_parameter("xt", [128, CCH, B], F32, isOutput=False)
    bt_d = nc.declare_dram_parameter("bt", [128, HT], F32, isOutput=False)
    ch_d = nc.declare_dram_parameter("ch", [128, B], F32, isOutput=False)
    q_d = nc.declare_dram_parameter("q", [B, 2, N], F32, isOutput=True)

    with tile.TileContext(nc) as tc, ExitStack() as ctx:
        pool = ctx.enter_context(tc.tile_pool(name="sbuf", bufs=1))
        psum = ctx.enter_context(
            tc.tile_pool(name="psum", bufs=1, space=bass.MemorySpace.PSUM)
        )

        wct = pool.tile([128, HT, N], F32, tag="wct")
        nc.sync.dma_start(wct[:], wct_d[:])
        wc = pool.tile([128, CCH, HL], F32, tag="wc")
        nc.sync.dma_start(wc[:], wc_d[:])
        xt = pool.tile([128, CCH, B], F32, tag="xt")
        nc.sync.dma_start(xt[:], xt_d[:])
        bt = pool.tile([128, HT], F32, tag="bt")
        nc.sync.dma_start(bt[:], bt_d[:])

        neg_half = pool.tile([128, B], F32, tag="neg_half")
        nc.sync.dma_start(neg_half[:], ch_d[:])

        # thetaT[h, b] = sum_n W[n, h] x[n, b]   (h on partitions)
        th = psum.tile([128, HT, B], F32, tag="th")
        for t in range(HT):
            for c in range(CCH):
                nc.tensor.matmul(
                    th[:, t, :],
                    wc[:, c, bass.ts(t, 128)],
                    xt[:, c, :],
                    start=(c == 0),
                    stop=(c == CCH - 1),
                )

        # t = tanh(thetaT + b); per-h-tile bias is per-partition
        T1 = pool.tile([128, HT, B], F32, tag="T1")
        for t in range(HT):
            nc.scalar.activation(
                T1[:, t, :], th[:, t, :], AF.Tanh, bias=bt[:, t : t + 1]
            )
        # T'_n = t^n / n (scaled so PSUM accumulates sum_n M_n/n directly)
        T2 = pool.tile([128, HT, B], F32, tag="T2")
        nc.vector.scalar_tensor_tensor(T2[:], T1[:], 0.5, T1[:], ALU.mult, ALU.mult)
        T3 = pool.tile([128, HT, B], F32R, tag="T3")
        nc.vector.scalar_tensor_tensor(
            T3[:], T2[:], 2.0 / 3.0, T1[:], ALU.mult, ALU.mult
        )
        T4 = pool.tile([128, HT, B], F32R, tag="T4")
        nc.vector.tensor_mul(T4[:], T2[:], T2[:])

        # Gamma_n = tanh(2W)^n, layout [h_p, k_f]
        G1 = pool.tile([128, HT, N], F32, tag="G1")
        nc.scalar.activation(G1[:], wct[:], AF.Tanh, scale=2.0)
        G2 = pool.tile([128, HT, N], F32, tag="G2")
        nc.scalar.activation(G2[:], G1[:], AF.Square)
        G4 = pool.tile([128, HT, N], F32R, tag="G4")
        nc.scalar.activation(G4[:], G2[:], AF.Square)
        G3 = pool.tile([128, HT, N], F32R, tag="G3")
        nc.vector.tensor_mul(G3[:], G2[:], G1[:])
        # U = G2 + G4/2; q_even gets (-1/2)*sum_h U = -S1 (2-term logcosh series)
        U = pool.tile([128, HT, N], F32, tag="U")
        nc.vector.scalar_tensor_tensor(U[:], G4[:], 0.5, G2[:], ALU.mult, ALU.add)

        # Odd-parity accumulation: M_1/1 + M_3/3
        qo = psum.tile([B, N], F32, tag="qo")
        odd_pairs = [(T1, G1), (T3, G3)]
        n_odd = len(odd_pairs) * HT
        k = 0
        for Tn, Gn in odd_pairs:
            for t in range(HT):
                nc.tensor.matmul(
                    qo[:],
                    Tn[:, t, :],
                    Gn[:, t, :],
                    start=(k == 0),
                    stop=(k == n_odd - 1),
                )
                k += 1

        # Even-parity accumulation: M_2/2 + M_4/4 - S1
        qe = psum.tile([B, N], F32, tag="qe")
        even_pairs = [(T2, G2), (T4, G4), (neg_half, U)]
        n_even = len(even_pairs) * HT
        k = 0
        for Tn, Gn in even_pairs:
            for t in range(HT):
                lhs = Tn[:] if len(Tn.shape) == 2 else Tn[:, t, :]
                nc.tensor.matmul(
                    qe[:],
                    lhs,
                    Gn[:, t, :],
                    start=(k == 0),
                    stop=(k == n_even - 1),
                )
                k += 1

        q_sb = pool.tile([B, 2, N], F32, tag="q_sb")
        nc.vector.tensor_copy(q_sb[:, 0, :], qe[:])
        nc.scalar.copy(q_sb[:, 1, :], qo[:])
        nc.sync.dma_start(q_d[:], q_sb[:])

    nc.compile()
    return nc


def _get_bass():
    if "nc" not in _CACHE:
        _CACHE["nc"] = _build_bass()
    return _CACHE["nc"]


def _prep_inputs(x, W, b, a):
    """Per-core input maps. All host-side layout prep, float32."""
    x = np.asarray(x, dtype=np.float32)
    W = np.asarray(W, dtype=np.float32)
    b = np.asarray(b, dtype=np.float32)

    xt = np.ascontiguousarray(
        x.T.reshape(CCH, 128, B).transpose(1, 0, 2)
    )  # [128, CCH, B]; xt[p, c, bb] = x[bb, c*128 + p]

    ch = np.full((128, B), -0.5, dtype=np.float32)
    in_maps = []
    for c in range(NCORES):
        sl = slice(c * HL, (c + 1) * HL)
        Wc = W[:, sl]  # [N, HL]
        wct = np.ascontiguousarray(
            Wc.T.reshape(HT, 128, N).transpose(1, 0, 2)
        )  # [128, HT, N]; wct[p, t, k] = W[k, c*HL + t*128 + p]
        wc = np.ascontiguousarray(
            Wc.reshape(CCH, 128, HL).transpose(1, 0, 2)
        )  # [128, CCH, HL]
        bt = np.ascontiguousarray(
            b[sl].reshape(HT, 128).T
        )  # [128, HT]
        in_maps.append({"wct": wct, "wc": wc, "xt": xt, "bt": bt, "ch": ch})
    return in_maps


def _combine(x, a, Oxy, results):
    q = np.zeros((B, 2, N), dtype=np.float32)
    for r in results:
        q += r["q"]
    x = np.asarray(x, dtype=np.float32)
    a = np.asarray(a, dtype=np.float32)
    Oxy = np.asarray(Oxy, dtype=np.float32)
    E = np.exp(-(q[:, 0, :] + x * (q[:, 1, :] + 2.0 * a)))
    return (E @ Oxy).astype(np.float32)


def kernel(x, W, b, a, Oxy):
    nc = _get_bass()
    in_maps = _prep_inputs(x, W, b, a)
    res = run_bass_kernel_spmd(nc, in_maps, list(range(NCORES))).results
    return _combine(x, a, Oxy, res)


# revision 17
# speedup vs baseline: 1.2325x; 1.0056x over previous
"""RBM local-operator kernel for Trainium2 (8 NeuronCores, SPMD).

Math: for y_k = x with spin k flipped (x in {-1,+1}^N),
  logpsi(y_k) - logpsi(x)
    = -2 x_k a_k + sum_h [ logcosh(th_h - 2 x_k W_kh) - logcosh(th_h) ]
    = -2 x_k a_k + S1_k + sum_h log(1 - x_k t_h tau_kh)
with th = xW + b, t = tanh(th), tau = tanh(2W), S1_k = sum_h logcosh(2W_kh).
Since |t*tau| <~ 0.08, log(1-u) = -sum_n u^n / n converges in 4 terms and each
term is a matmul over h:  sum_h t^n tau^n = (t^n) @ (tau^n)^T.  Also
logcosh(2W) = -0.5*log(1 - tau^2) ~= 0.5*(v + v^2/2), v = tau^2, which folds
into the same PSUM accumulation via a constant (-0.5) lhsT tile.

Sharding: hidden dim H=2048 split over 8 cores (256 each). Each core emits
  q_even = -S1 + sum_{n in 2,4} M_n / n
  q_odd  =       sum_{n in 1,3} M_n / n
Host combines: out = exp(-(sum_c q_even + x * (sum_c q_odd + 2a))) @ Oxy.

Matmuls run as float32r (single-pass fp32, 4x faster than fp32 for moving
dim >= 256) via bitcast views of the fp32 tiles.
"""

import sys

import numpy as np

_BASS_REPO = "/opt/trn_rl_repo"
if _BASS_REPO not in sys.path:
    sys.path.insert(0, _BASS_REPO)

from contextlib import ExitStack

import concourse.bass as bass
import concourse.tile as tile
from concourse import bacc, mybir
from concourse.bass_utils import run_bass_kernel_spmd

B, N, H, NCORES = 64, 512, 2048, 8
HL = H // NCORES          # hidden slice per core: 256
HT = HL // 128            # SBUF partition tiles per slice: 2
CCH = N // 128            # theta contraction chunks: 4
F32 = mybir.dt.float32
F32R = mybir.dt.float32r
BF16 = mybir.dt.bfloat16
AF = mybir.ActivationFunctionType
ALU = mybir.AluOpType

_CACHE = {}


def _r(ap):
    """View an fp32 AP as float32r for single-pass PE matmul."""
    return ap.bitcast(F32R)


def _build_bass():
    nc = bacc.Bacc(
        "TRN2", target_bir_lowering=False, debug=False, num_devices=NCORES
    )
    wct_d = nc.declare_dram_parameter("wct", [128, HT, N], F32, isOutput=False)
    wch_d = nc.declare_dram_parameter("wch", [128, CCH, HL], BF16, isOutput=False)
    wcl_d = nc.declare_dram_parameter("wcl", [128, CCH, HL], BF16, isOutput=False)
    xtb_d = nc.declare_dram_parameter("xtb", [128, CCH, B], BF16, isOutput=False)
    bt_d = nc.declare_dram_parameter("bt", [128, HT], F32, isOutput=False)
    ch_d = nc.declare_dram_parameter("ch", [128, B], F32, isOutput=False)
    q_d = nc.declare_dram_parameter("q", [B, 2, N], F32, isOutput=True)

    with tile.TileContext(nc) as tc, ExitStack() as ctx:
        pool = ctx.enter_context(tc.tile_pool(name="sbuf", bufs=1))
        psum = ctx.enter_context(
            tc.tile_pool(name="psum", bufs=1, space=bass.MemorySpace.PSUM)
        )

        wct = pool.tile([128, HT, N], F32, tag="wct")
        nc.sync.dma_start(wct[:], wct_d[:])
        wch = pool.tile([128, CCH, HL], BF16, tag="wch")
        nc.gpsimd.dma_start(wch[:], wch_d[:])
        wcl = pool.tile([128, CCH, HL], BF16, tag="wcl")
        nc.gpsimd.dma_start(wcl[:], wcl_d[:])
        xtb = pool.tile([128, CCH, B], BF16, tag="xtb")
        nc.scalar.dma_start(xtb[:], xtb_d[:])
        bt = pool.tile([128, HT], F32, tag="bt")
        nc.scalar.dma_start(bt[:], bt_d[:])

        neg_half = pool.tile([128, B], F32, tag="neg_half")
        nc.scalar.dma_start(neg_half[:], ch_d[:])

        # PE warm-up: zero-contribution matmuls (rhs = 0) into the qo bank.
        # Keeps the PE busy ~3.4us so the HAM clock gate opens to 2.4 GHz
        # before the real matmuls; sets has_written across qo.
        zz = pool.tile([128, N], F32, tag="zz")
        nc.vector.memset(zz[:], 0.0)

        # warm-up group opens the qo accumulation (contributes exactly 0)
        qo = psum.tile([B, N], F32, tag="qo")
        N_WARM = 2
        for i in range(N_WARM):
            nc.tensor.matmul(qo[:], neg_half[:], zz[:], start=(i == 0), stop=False)

        # thetaT[h, b] = sum_n W[n, h] x[n, b]   (h on partitions)
        # exact via bf16 split: W = Whi + Wlo, x is +-1 (exact in bf16)
        th = psum.tile([128, HT, B], F32, tag="th")
        for t in range(HT):
            n_th = 2 * CCH
            k = 0
            for c in range(CCH):
                for wpart in (wch, wcl):
                    nc.tensor.matmul(
                        th[:, t, :],
                        wpart[:, c, bass.ts(t, 128)],
                        xtb[:, c, :],
                        start=(k == 0),
                        stop=(k == n_th - 1),
                    )
                    k += 1

        # t = tanh(thetaT + b); per-h-tile bias is per-partition
        T1 = pool.tile([128, HT, B], F32, tag="T1")
        for t in range(HT):
            nc.scalar.activation(
                T1[:, t, :], th[:, t, :], AF.Tanh, bias=bt[:, t : t + 1]
            )
        # T'_n = t^n / n (scaled so PSUM accumulates sum_n M_n/n directly)
        T2 = pool.tile([128, HT, B], F32, tag="T2")
        nc.vector.scalar_tensor_tensor(T2[:], T1[:], 0.5, T1[:], ALU.mult, ALU.mult)
        T3 = pool.tile([128, HT, B], F32R, tag="T3")
        nc.vector.scalar_tensor_tensor(
            T3[:], T2[:], 2.0 / 3.0, T1[:], ALU.mult, ALU.mult
        )
        T4 = pool.tile([128, HT, B], F32R, tag="T4")
        nc.vector.tensor_mul(T4[:], T2[:], T2[:])

        # Gamma_n = tanh(2W)^n, layout [h_p, k_f]
        G1 = pool.tile([128, HT, N], F32, tag="G1")
        nc.scalar.activation(G1[:], wct[:], AF.Tanh, scale=2.0)
        G2 = pool.tile([128, HT, N], F32, tag="G2")
        nc.scalar.activation(G2[:], G1[:], AF.Square)
        G4 = pool.tile([128, HT, N], F32R, tag="G4")
        nc.scalar.activation(G4[:], G2[:], AF.Square)
        G3 = pool.tile([128, HT, N], F32R, tag="G3")
        nc.vector.tensor_mul(G3[:], G2[:], G1[:])
        # U = G2 + G4/2; q_even gets (-1/2)*sum_h U = -S1 (2-term logcosh series)
        U = pool.tile([128, HT, N], F32, tag="U")
        nc.vector.scalar_tensor_tensor(U[:], G4[:], 0.5, G2[:], ALU.mult, ALU.add)

        # Odd-parity accumulation: M_1/1 + M_3/3 (bank opened by warm-up)
        odd_pairs = [(T1, G1), (T3, G3)]
        n_odd = len(odd_pairs) * HT
        k = 0
        for Tn, Gn in odd_pairs:
            for t in range(HT):
                nc.tensor.matmul(
                    qo[:],
                    Tn[:, t, :],
                    Gn[:, t, :],
                    start=False,
                    stop=(k == n_odd - 1),
                )
                k += 1

        # Even-parity accumulation: M_2/2 + M_4/4 - S1
        qe = psum.tile([B, N], F32, tag="qe")
        even_pairs = [(T2, G2), (T4, G4), (neg_half, U)]
        n_even = len(even_pairs) * HT
        k = 0
        for Tn, Gn in even_pairs:
            for t in range(HT):
                lhs = Tn[:] if len(Tn.shape) == 2 else Tn[:, t, :]
                nc.tensor.matmul(
                    qe[:],
                    lhs,
                    Gn[:, t, :],
                    start=(k == 0),
                    stop=(k == n_even - 1),
                )
                k += 1

        q_sb = pool.tile([B, 2, N], F32, tag="q_sb")
        nc.vector.tensor_copy(q_sb[:, 0, :], qe[:])
        nc.scalar.copy(q_sb[:, 1, :], qo[:])
        nc.sync.dma_start(q_d[:], q_sb[:])

    nc.compile()
    return nc


def _get_bass():
    if "nc" not in _CACHE:
        _CACHE["nc"] = _build_bass()
    return _CACHE["nc"]


def _prep_inputs(x, W, b, a):
    """Per-core input maps. All host-side layout prep, float32."""
    import ml_dtypes

    bf16 = ml_dtypes.bfloat16
    x = np.asarray(x, dtype=np.float32)
    W = np.asarray(W, dtype=np.float32)
    b = np.asarray(b, dtype=np.float32)

    xtb = np.ascontiguousarray(
        x.T.reshape(CCH, 128, B).transpose(1, 0, 2)
    ).astype(bf16)  # [128, CCH, B]; xt[p, c, bb] = x[bb, c*128 + p]

    ch = np.full((128, B), -0.5, dtype=np.float32)
    in_maps = []
    for c in range(NCORES):
        sl = slice(c * HL, (c + 1) * HL)
        Wc = W[:, sl]  # [N, HL]
        wct = np.ascontiguousarray(
            Wc.T.reshape(HT, 128, N).transpose(1, 0, 2)
        )  # [128, HT, N]; wct[p, t, k] = W[k, c*HL + t*128 + p]
        wc = np.ascontiguousarray(
            Wc.reshape(CCH, 128, HL).transpose(1, 0, 2)
        )  # [128, CCH, HL]
        wch = wc.astype(bf16)
        wcl = (wc - wch.astype(np.float32)).astype(bf16)
        bt = np.ascontiguousarray(
            b[sl].reshape(HT, 128).T
        )  # [128, HT]
        in_maps.append(
            {"wct": wct, "wch": wch, "wcl": wcl, "xtb": xtb, "bt": bt, "ch": ch}
        )
    return in_maps


def _combine(x, a, Oxy, results):
    q = np.zeros((B, 2, N), dtype=np.float32)
    for r in results:
        q += r["q"]
    x = np.asarray(x, dtype=np.float32)
    a = np.asarray(a, dtype=np.float32)
    Oxy = np.asarray(Oxy, dtype=np.float32)
    E = np.exp(-(q[:, 0, :] + x * (q[:, 1, :] + 2.0 * a)))
    return (E @ Oxy).astype(np.float32)


def kernel(x, W, b, a, Oxy):
    nc = _get_bass()
    in_maps = _prep_inputs(x, W, b, a)
    res = run_bass_kernel_spmd(nc, in_maps, list(range(NCORES))).results
    return _combine(x, a, Oxy, res)


# revision 18
# speedup vs baseline: 1.3174x; 1.0689x over previous
"""RBM local-operator kernel for Trainium2 (8 NeuronCores, SPMD).

Math: for y_k = x with spin k flipped (x in {-1,+1}^N),
  logpsi(y_k) - logpsi(x)
    = -2 x_k a_k + S1_k + sum_h log(1 - x_k t_h tau_kh)
with th = xW + b, t = tanh(th), tau = tanh(2W), S1_k = sum_h logcosh(2W_kh).
Since |t*tau| <~ 0.08, log(1-u) = -sum_n u^n/n converges in 4 terms; each term
is a matmul over h: sum_h t^n tau^n = (t^n) @ (tau^n)^T.  Also
logcosh(2W) = -0.5*log(1 - tau^2) ~= 0.5*(tau^2 + tau^4/2), folded into the
same PSUM accumulation via constant lhsT tiles (-1/2 on tau^2, -1/4 on tau^4).

Sharding: hidden dim H=2048 split over 8 cores (256 each). Each core emits
  q_even = -S1 + M_2/2 + M_4/4      q_odd = M_1 + M_3/3
Host combines: out = exp(-(sum_c q_even + x * (sum_c q_odd + 2a))) @ Oxy.

Precision: theta via exact bf16 hi/lo split (x is +-1, exact in bf16);
n=1,2 terms and the tau^2 S1 term in fp32; n=3,4 and the tau^4 S1 term in
float32r (single-pass PE).  A zero-contribution matmul burst (rhs = 0) warms
the PE clock gate (HAM) before the real matmuls.
"""

import sys

import numpy as np

_BASS_REPO = "/opt/trn_rl_repo"
if _BASS_REPO not in sys.path:
    sys.path.insert(0, _BASS_REPO)

from contextlib import ExitStack

import concourse.bass as bass
import concourse.tile as tile
from concourse import bacc, mybir
from concourse.bass_utils import run_bass_kernel_spmd

B, N, H, NCORES = 64, 512, 2048, 8
HL = H // NCORES          # hidden slice per core: 256
HT = HL // 128            # SBUF partition tiles per slice: 2
CCH = N // 128            # theta contraction chunks: 4
F32 = mybir.dt.float32
F32R = mybir.dt.float32r
BF16 = mybir.dt.bfloat16
AF = mybir.ActivationFunctionType
ALU = mybir.AluOpType

_CACHE = {}


def _build_bass():
    nc = bacc.Bacc(
        "TRN2", target_bir_lowering=False, debug=False, num_devices=NCORES
    )
    wct_d = nc.declare_dram_parameter("wct", [128, HT, N], F32, isOutput=False)
    wch_d = nc.declare_dram_parameter("wch", [128, CCH, HL], BF16, isOutput=False)
    wcl_d = nc.declare_dram_parameter("wcl", [128, CCH, HL], BF16, isOutput=False)
    xtb_d = nc.declare_dram_parameter("xtb", [128, CCH, B], BF16, isOutput=False)
    bt_d = nc.declare_dram_parameter("bt", [128, HT], F32, isOutput=False)
    ch_d = nc.declare_dram_parameter("ch", [128, B], F32, isOutput=False)
    cq_d = nc.declare_dram_parameter("cq", [128, B], F32R, isOutput=False)
    q_d = nc.declare_dram_parameter("q", [B, 2, N], F32, isOutput=True)

    with tile.TileContext(nc) as tc, ExitStack() as ctx:
        pool = ctx.enter_context(tc.tile_pool(name="sbuf", bufs=1))
        psum = ctx.enter_context(
            tc.tile_pool(name="psum", bufs=1, space=bass.MemorySpace.PSUM)
        )

        # All input DMAs on the SP (sync) HWDGE ring; wct first (gates the
        # ACT Gamma chain, the longest pole).
        wct = pool.tile([128, HT, N], F32, tag="wct")
        nc.sync.dma_start(wct[:], wct_d[:])
        wch = pool.tile([128, CCH, HL], BF16, tag="wch")
        nc.sync.dma_start(wch[:], wch_d[:])
        wcl = pool.tile([128, CCH, HL], BF16, tag="wcl")
        nc.sync.dma_start(wcl[:], wcl_d[:])
        xtb = pool.tile([128, CCH, B], BF16, tag="xtb")
        nc.sync.dma_start(xtb[:], xtb_d[:])
        bt = pool.tile([128, HT], F32, tag="bt")
        nc.sync.dma_start(bt[:], bt_d[:])
        neg_half = pool.tile([128, B], F32, tag="neg_half")
        nc.sync.dma_start(neg_half[:], ch_d[:])
        neg_quart = pool.tile([128, B], F32R, tag="neg_quart")
        nc.sync.dma_start(neg_quart[:], cq_d[:])

        zz = pool.tile([128, N], F32, tag="zz")
        nc.vector.memset(zz[:], 0.0)

        # PE warm-up: zero-contribution matmuls (rhs = 0) into the qo bank.
        # Spins the PE ~3.4us so the HAM clock gate opens to 2.4 GHz before
        # the real matmuls; start=True sets has_written across the bank.
        qo = psum.tile([B, N], F32, tag="qo")
        N_WARM = 2
        for i in range(N_WARM):
            nc.tensor.matmul(qo[:], zz[:, :B], zz[:], start=(i == 0), stop=False)

        # thetaT[h, b] = sum_n W[n, h] x[n, b]   (h on partitions)
        # exact via bf16 split: W = Whi + Wlo, x is +-1 (exact in bf16)
        th = psum.tile([128, HT, B], F32, tag="th")
        for t in range(HT):
            n_th = 2 * CCH
            k = 0
            for c in range(CCH):
                for wpart in (wch, wcl):
                    nc.tensor.matmul(
                        th[:, t, :],
                        wpart[:, c, bass.ts(t, 128)],
                        xtb[:, c, :],
                        start=(k == 0),
                        stop=(k == n_th - 1),
                    )
                    k += 1

        # Gamma chain on ACT: G1 -> G2, then t, then G4
        G1 = pool.tile([128, HT, N], F32, tag="G1")
        nc.scalar.activation(G1[:], wct[:], AF.Tanh, scale=2.0)
        G2 = pool.tile([128, HT, N], F32, tag="G2")
        nc.scalar.activation(G2[:], G1[:], AF.Square)

        T1 = pool.tile([128, HT, B], F32, tag="T1")
        for t in range(HT):
            nc.scalar.activation(
                T1[:, t, :], th[:, t, :], AF.Tanh, bias=bt[:, t : t + 1]
            )

        G4 = pool.tile([128, HT, N], F32R, tag="G4")
        nc.scalar.activation(G4[:], G2[:], AF.Square)
        G3 = pool.tile([128, HT, N], F32R, tag="G3")
        nc.vector.tensor_mul(G3[:], G2[:], G1[:])

        # T'_n = t^n / n on DVE
        T2 = pool.tile([128, HT, B], F32, tag="T2")
        nc.vector.scalar_tensor_tensor(T2[:], T1[:], 0.5, T1[:], ALU.mult, ALU.mult)
        T3 = pool.tile([128, HT, B], F32R, tag="T3")
        nc.vector.scalar_tensor_tensor(
            T3[:], T2[:], 2.0 / 3.0, T1[:], ALU.mult, ALU.mult
        )
        T4 = pool.tile([128, HT, B], F32R, tag="T4")
        nc.vector.tensor_mul(T4[:], T2[:], T2[:])

        # Odd bank (opened by warm-up): M_1 + M_3/3
        for t in range(HT):
            nc.tensor.matmul(qo[:], T1[:, t, :], G1[:, t, :], start=False, stop=False)
        for t in range(HT):
            nc.tensor.matmul(
                qo[:], T3[:, t, :], G3[:, t, :], start=False, stop=(t == HT - 1)
            )

        # Even bank: M_2/2 - S1_tau2 + M_4/4 - S1_tau4
        qe = psum.tile([B, N], F32, tag="qe")
        for t in range(HT):
            nc.tensor.matmul(
                qe[:], T2[:, t, :], G2[:, t, :], start=(t == 0), stop=False
            )
        for t in range(HT):
            nc.tensor.matmul(qe[:], neg_half[:], G2[:, t, :], start=False, stop=False)
        for t in range(HT):
            nc.tensor.matmul(qe[:], T4[:, t, :], G4[:, t, :], start=False, stop=False)
        for t in range(HT):
            nc.tensor.matmul(
                qe[:], neg_quart[:], G4[:, t, :], start=False, stop=(t == HT - 1)
            )

        q_sb = pool.tile([B, 2, N], F32, tag="q_sb")
        nc.scalar.copy(q_sb[:, 1, :], qo[:])
        nc.vector.tensor_copy(q_sb[:, 0, :], qe[:])
        nc.sync.dma_start(q_d[:], q_sb[:])

    nc.compile()
    return nc


def _get_bass():
    if "nc" not in _CACHE:
        _CACHE["nc"] = _build_bass()
    return _CACHE["nc"]


def _prep_inputs(x, W, b, a):
    """Per-core input maps. All host-side layout prep."""
    import ml_dtypes

    bf16 = ml_dtypes.bfloat16
    x = np.asarray(x, dtype=np.float32)
    W = np.asarray(W, dtype=np.float32)
    b = np.asarray(b, dtype=np.float32)

    xtb = np.ascontiguousarray(
        x.T.reshape(CCH, 128, B).transpose(1, 0, 2)
    ).astype(bf16)  # [128, CCH, B]; xt[p, c, bb] = x[bb, c*128 + p]

    ch = np.full((128, B), -0.5, dtype=np.float32)
    cq = np.full((128, B), -0.25, dtype=np.float32)
    in_maps = []
    for c in range(NCORES):
        sl = slice(c * HL, (c + 1) * HL)
        Wc = W[:, sl]  # [N, HL]
        wct = np.ascontiguousarray(
            Wc.T.reshape(HT, 128, N).transpose(1, 0, 2)
        )  # [128, HT, N]; wct[p, t, k] = W[k, c*HL + t*128 + p]
        wc = np.ascontiguousarray(
            Wc.reshape(CCH, 128, HL).transpose(1, 0, 2)
        )  # [128, CCH, HL]
        wch = wc.astype(bf16)
        wcl = (wc - wch.astype(np.float32)).astype(bf16)
        bt = np.ascontiguousarray(b[sl].reshape(HT, 128).T)  # [128, HT]
        in_maps.append(
            {"wct": wct, "wch": wch, "wcl": wcl, "xtb": xtb, "bt": bt,
             "ch": ch, "cq": cq}
        )
    return in_maps


def _combine(x, a, Oxy, results):
    q = np.zeros((B, 2, N), dtype=np.float32)
    for r in results:
        q += r["q"]
    x = np.asarray(x, dtype=np.float32)
    a = np.asarray(a, dtype=np.float32)
    Oxy = np.asarray(Oxy, dtype=np.float32)
    E = np.exp(-(q[:, 0, :] + x * (q[:, 1, :] + 2.0 * a)))
    return (E @ Oxy).astype(np.float32)


def kernel(x, W, b, a, Oxy):
    nc = _get_bass()
    in_maps = _prep_inputs(x, W, b, a)
    res = run_bass_kernel_spmd(nc, in_maps, list(range(NCORES))).results
    return _combine(x, a, Oxy, res)


# revision 19
# speedup vs baseline: 1.3678x; 1.0383x over previous
"""RBM local-operator kernel for Trainium2 (8 NeuronCores, SPMD).

Math: for y_k = x with spin k flipped (x in {-1,+1}^N),
  logpsi(y_k) - logpsi(x)
    = -2 x_k a_k + S1_k + sum_h log(1 - x_k t_h tau_kh)
with th = xW + b, t = tanh(th), tau = tanh(2W), S1_k = sum_h logcosh(2W_kh).
Since |t*tau| <~ 0.08, log(1-u) = -sum_n u^n/n converges in 4 terms; each term
is a matmul over h: sum_h t^n tau^n = (t^n) @ (tau^n)^T.  Also
logcosh(2W) = -0.5*log(1 - tau^2) ~= 0.5*(tau^2 + tau^4/2), folded into the
same PSUM accumulation via constant lhsT tiles (-1/2 on tau^2, -1/4 on tau^4).

Sharding: hidden dim H=2048 split over 8 cores (256 each). Each core emits
  q_even = -S1 + M_2/2 + M_4/4      q_odd = M_1 + M_3/3
Host combines: out = exp(-(sum_c q_even + x * (sum_c q_odd + 2a))) @ Oxy.

Precision: theta via exact bf16 hi/lo split (x is +-1, exact in bf16);
n=1,2 terms and the tau^2 S1 term in fp32; n=3,4 and the tau^4 S1 term in
float32r (single-pass PE).  A zero-contribution matmul burst (rhs = 0) warms
the PE clock gate (HAM) before the real matmuls.
"""

import sys

import numpy as np

_BASS_REPO = "/opt/trn_rl_repo"
if _BASS_REPO not in sys.path:
    sys.path.insert(0, _BASS_REPO)

from contextlib import ExitStack

import concourse.bass as bass
import concourse.tile as tile
from concourse import bacc, mybir
from concourse.bass_utils import run_bass_kernel_spmd

B, N, H, NCORES = 64, 512, 2048, 8
HL = H // NCORES          # hidden slice per core: 256
HT = HL // 128            # SBUF partition tiles per slice: 2
CCH = N // 128            # theta contraction chunks: 4
F32 = mybir.dt.float32
F32R = mybir.dt.float32r
BF16 = mybir.dt.bfloat16
AF = mybir.ActivationFunctionType
ALU = mybir.AluOpType

_CACHE = {}


def _build_bass():
    nc = bacc.Bacc(
        "TRN2", target_bir_lowering=False, debug=False, num_devices=NCORES
    )
    wct_d = nc.declare_dram_parameter("wct", [128, HT, N], F32, isOutput=False)
    wpk_d = nc.declare_dram_parameter(
        "wpk", [128, CCH, 2 * HL + B], BF16, isOutput=False
    )
    spk_d = nc.declare_dram_parameter("spk", [128, HT + B], F32, isOutput=False)
    cq_d = nc.declare_dram_parameter("cq", [128, B], F32R, isOutput=False)
    q_d = nc.declare_dram_parameter("q", [B, 2, N], F32, isOutput=True)

    with tile.TileContext(nc) as tc, ExitStack() as ctx:
        pool = ctx.enter_context(tc.tile_pool(name="sbuf", bufs=1))
        psum = ctx.enter_context(
            tc.tile_pool(name="psum", bufs=1, space=bass.MemorySpace.PSUM)
        )

        # All input DMAs on the SP (sync) HWDGE ring; wct first (gates the
        # ACT Gamma chain, the longest pole).
        wct = pool.tile([128, HT, N], F32, tag="wct")
        nc.sync.dma_start(wct[:], wct_d[:])
        wpk = pool.tile([128, CCH, 2 * HL + B], BF16, tag="wpk")
        nc.sync.dma_start(wpk[:], wpk_d[:])
        spk = pool.tile([128, HT + B], F32, tag="spk")
        nc.scalar.dma_start(spk[:], spk_d[:])
        neg_half2 = pool.tile([128, B], F32R, tag="neg_half2")
        nc.scalar.dma_start(neg_half2[:], cq_d[:])
        bt = spk[:, 0:HT]
        neg_half = spk[:, HT : HT + B]

        zz = pool.tile([128, N], F32, tag="zz")
        nc.vector.memset(zz[:], 0.0)

        # PE warm-up: zero-contribution matmuls (rhs = 0) into the qo bank.
        # Spins the PE ~3.4us so the HAM clock gate opens to 2.4 GHz before
        # the real matmuls; start=True sets has_written across the bank.
        qo = psum.tile([B, N], F32, tag="qo")
        N_WARM = 1
        for i in range(N_WARM):
            nc.tensor.matmul(qo[:], zz[:, :B], zz[:], start=(i == 0), stop=False)

        # thetaT[h, b] = sum_n W[n, h] x[n, b]   (h on partitions)
        # exact via bf16 split: W = Whi + Wlo, x is +-1 (exact in bf16)
        th = psum.tile([128, HT, B], F32, tag="th")
        for t in range(HT):
            n_th = 2 * CCH
            k = 0
            for c in range(CCH):
                for off in (0, HL):
                    nc.tensor.matmul(
                        th[:, t, :],
                        wpk[:, c, off + t * 128 : off + (t + 1) * 128],
                        wpk[:, c, 2 * HL : 2 * HL + B],
                        start=(k == 0),
                        stop=(k == n_th - 1),
                    )
                    k += 1

        # Gamma chain on ACT: G1 -> G2, then t, then G4
        G1 = pool.tile([128, HT, N], F32, tag="G1")
        nc.scalar.activation(G1[:], wct[:], AF.Tanh, scale=2.0)
        G2 = pool.tile([128, HT, N], F32, tag="G2")
        nc.scalar.activation(G2[:], G1[:], AF.Square)

        T1 = pool.tile([128, HT, B], F32, tag="T1")
        for t in range(HT):
            nc.scalar.activation(
                T1[:, t, :], th[:, t, :], AF.Tanh, bias=bt[:, t : t + 1]
            )

        G4 = pool.tile([128, HT, N], F32R, tag="G4")
        nc.scalar.activation(G4[:], G2[:], AF.Square, scale=0.7071067811865476)
        G3 = pool.tile([128, HT, N], F32R, tag="G3")
        nc.vector.tensor_mul(G3[:], G2[:], G1[:])

        # T'_n = t^n / n on DVE
        T2 = pool.tile([128, HT, B], F32, tag="T2")
        nc.vector.scalar_tensor_tensor(T2[:], T1[:], 0.5, T1[:], ALU.mult, ALU.mult)
        T3 = pool.tile([128, HT, B], F32R, tag="T3")
        nc.vector.scalar_tensor_tensor(
            T3[:], T2[:], 2.0 / 3.0, T1[:], ALU.mult, ALU.mult
        )
        T4 = pool.tile([128, HT, B], F32R, tag="T4")
        nc.vector.scalar_tensor_tensor(T4[:], T2[:], 2.0, T2[:], ALU.mult, ALU.mult)

        # Odd bank (opened by warm-up): M_1 + M_3/3
        for t in range(HT):
            nc.tensor.matmul(qo[:], T1[:, t, :], G1[:, t, :], start=False, stop=False)
        for t in range(HT):
            nc.tensor.matmul(
                qo[:], T3[:, t, :], G3[:, t, :], start=False, stop=(t == HT - 1)
            )

        # Even bank: M_2/2 - S1_tau2 + M_4/4 - S1_tau4
        qe = psum.tile([B, N], F32, tag="qe")
        for t in range(HT):
            nc.tensor.matmul(
                qe[:], T2[:, t, :], G2[:, t, :], start=(t == 0), stop=False
            )
        for t in range(HT):
            nc.tensor.matmul(qe[:], neg_half[:], G2[:, t, :], start=False, stop=False)
        for t in range(HT):
            nc.tensor.matmul(qe[:], T4[:, t, :], G4[:, t, :], start=False, stop=False)
        for t in range(HT):
            nc.tensor.matmul(
                qe[:], neg_half2[:], G4[:, t, :], start=False, stop=(t == HT - 1)
            )

        q_sb = pool.tile([B, 2, N], F32, tag="q_sb")
        nc.scalar.copy(q_sb[:, 1, :], qo[:])
        nc.vector.tensor_copy(q_sb[:, 0, :], qe[:])
        nc.sync.dma_start(q_d[:], q_sb[:])

    nc.compile()
    return nc


def _get_bass():
    if "nc" not in _CACHE:
        _CACHE["nc"] = _build_bass()
    return _CACHE["nc"]


def _prep_inputs(x, W, b, a):
    """Per-core input maps. All host-side layout prep."""
    import ml_dtypes

    bf16 = ml_dtypes.bfloat16
    x = np.asarray(x, dtype=np.float32)
    W = np.asarray(W, dtype=np.float32)
    b = np.asarray(b, dtype=np.float32)

    xtb = np.ascontiguousarray(
        x.T.reshape(CCH, 128, B).transpose(1, 0, 2)
    ).astype(bf16)  # [128, CCH, B]; xt[p, c, bb] = x[bb, c*128 + p]

    cq = np.full((128, B), -0.5, dtype=np.float32)
    in_maps = []
    for c in range(NCORES):
        sl = slice(c * HL, (c + 1) * HL)
        Wc = W[:, sl]  # [N, HL]
        wct = np.ascontiguousarray(
            Wc.T.reshape(HT, 128, N).transpose(1, 0, 2)
        )  # [128, HT, N]; wct[p, t, k] = W[k, c*HL + t*128 + p]
        wc = np.ascontiguousarray(
            Wc.reshape(CCH, 128, HL).transpose(1, 0, 2)
        )  # [128, CCH, HL]
        wch = wc.astype(bf16)
        wcl = (wc - wch.astype(np.float32)).astype(bf16)
        wpk = np.empty((128, CCH, 2 * HL + B), dtype=bf16)
        wpk[:, :, 0:HL] = wch
        wpk[:, :, HL : 2 * HL] = wcl
        wpk[:, :, 2 * HL :] = xtb
        bt = np.ascontiguousarray(b[sl].reshape(HT, 128).T)  # [128, HT]
        spk = np.empty((128, HT + B), dtype=np.float32)
        spk[:, 0:HT] = bt
        spk[:, HT:] = -0.5
        in_maps.append({"wct": wct, "wpk": wpk, "spk": spk, "cq": cq})
    return in_maps


def _combine(x, a, Oxy, results):
    q = np.zeros((B, 2, N), dtype=np.float32)
    for r in results:
        q += r["q"]
    x = np.asarray(x, dtype=np.float32)
    a = np.asarray(a, dtype=np.float32)
    Oxy = np.asarray(Oxy, dtype=np.float32)
    E = np.exp(-(q[:, 0, :] + x * (q[:, 1, :] + 2.0 * a)))
    return (E @ Oxy).astype(np.float32)


def kernel(x, W, b, a, Oxy):
    nc = _get_bass()
    in_maps = _prep_inputs(x, W, b, a)
    res = run_bass_kernel_spmd(nc, in_maps, list(range(NCORES))).results
    return _combine(x, a, Oxy, res)
